# revision 8
# baseline (speedup 1.0000x reference)
"""Trainium2 Bass kernel for CausalSelfAttention (B=4, T=2048, C=1024, H=16)
with additive prev-prob key bias.

Sharding: 8 cores = data-parallel over B (4) x tensor-parallel over head
halves (2).  Each core computes qkv for its 8 heads, causal attention, and a
partial output projection (row-parallel W_proj); host sums the two partials
per batch at unshard time.

v2 design notes (vs the f32r baseline):
  - All matmul operands are fp16 (PSUM accumulation stays f32): halves DMA
    and SBUF traffic, enables FWL weight loads, 2x DVE rate. Measured CPU
    emulation rel-err ~5e-4 (gate is 2e-2).
  - The additive prev-prob bias rides as a per-partition (per-key) bias on
    the exp activation: se = exp(score/8 + 0.1*(-log(p+1e-10))), so V needs
    no per-key scaling and the softmax-denominator column of the PV
    stationary is a constant ones column (memset once).
  - Head B of each pair uses a [128,128] PV stationary (ones col 0 ->
    partition 0 = denominator, dims cols 64:128 -> partitions 64:128) so the
    normalize multiply writes stack[64:128] directly from PSUM -- no SBUF
    assembly DMA and no scalar-engine evacuation copies.
  - Denominator: reciprocal_approx_fast straight off the PSUM row (1-lane
    DVE, ~0.6us), one DMA to DRAM, two broadcast DMAs back (partition
    broadcast requires a DRAM source). ~2.5us chain vs ~9us in the baseline,
    which was the main source of PE stalls + HAM re-throttles.
  - 16 warm-up matmuls on memset tiles keep the PE HAM un-throttled through
    the initial DMA phase; chunk-0 prework matmuls are emitted c-innermost
    so they trickle behind the interleaved xs/wqkv DMA stream.
  - Filler scheduling: next-chunk QKV prework and prev-chunk output
    projection are pulled between a slot's QK and PV (covering the exp
    latency) with an adaptive rate; proj fillers are held back a few slots
    so their LDWEIGHTS never enters the PE queue before the stacks exist.
"""

import math
from contextlib import ExitStack

import numpy as np

import concourse.bass as bass
import concourse.tile as tile
from concourse import bacc, mybir

F32 = mybir.dt.float32
F16 = mybir.dt.float16

B, T, C, H = 4, 2048, 1024, 16
HD = C // H          # 64
NCORES = 8
HPC = H // 2         # 8 heads per core
FPC = HPC * HD       # 512 features per core
NKT = T // 128       # 16 key tiles
NQC = T // 512       # 4 query chunks (also the x t-chunks)
NCT = C // 128       # 8 contraction tiles
NP = HPC // 2        # 4 head pairs
EPS_BIAS = 0.1
SCALE = 1.0 / math.sqrt(HD)


def build(tc, out_ap, xT, wqkv, wproj, biasv, tri_dram, dsc):
    """Emit the per-core kernel into TileContext tc.

    out_ap : (T, C)    fp16 partial projection output (pair-summed on host)
    xT     : (C, T)    fp16 x[b] transposed
    wqkv   : (C, 3*FPC) fp16 [Wq_g | Wk_g | Wv_g] columns for this head group
    wproj  : (FPC, C)  fp16 W_proj rows for this head group
    biasv  : (T,)      f32 0.1 * -log(prev_probs[b] + 1e-10)
    tri_dram: (128,128) fp16 upper-triangular ones (tri[k,q] = 1 iff k <= q)
    dsc    : (16, 2048) f32 DRAM scratch for the denominator shuffle
    """
    nc = tc.nc
    ctx = tc.ctx
    Exp = mybir.ActivationFunctionType.Exp

    const = ctx.enter_context(tc.tile_pool(name="const", bufs=1))
    xs_pool = ctx.enter_context(tc.tile_pool(name="xs", bufs=9))
    qt_pool = ctx.enter_context(tc.tile_pool(name="qt", bufs=9))
    se_pool = ctx.enter_context(tc.tile_pool(name="se", bufs=3))
    rr_pool = ctx.enter_context(tc.tile_pool(name="rr", bufs=2))
    sc_pool = ctx.enter_context(tc.tile_pool(name="sc", bufs=2))
    stack_pool = ctx.enter_context(tc.tile_pool(name="stack", bufs=8))
    pout_pool = ctx.enter_context(tc.tile_pool(name="pout", bufs=3))

    ps_pool = ctx.enter_context(tc.tile_pool(name="ps", bufs=2, space="PSUM"))
    st_pool = ctx.enter_context(tc.tile_pool(name="st", bufs=2, space="PSUM"))
    y_pool = ctx.enter_context(tc.tile_pool(name="y", bufs=2, space="PSUM"))

    # ---- persistent buffers ----
    wq_sb = const.tile([128, NCT, 3 * FPC], F16, name="wq_sb")      # 24KB/p
    wp_sb = const.tile([128, FPC // 128, C], F16, name="wp_sb")     # 8KB/p
    kt = const.tile([128, NP, T], F16, name="kt")                   # 16KB/p
    v2a = const.tile([128, NKT, NP, HD + 1], F16, name="v2a")       # 8.1KB/p
    v2b = const.tile([128, NKT, NP, 128], F16, name="v2b")          # 16KB/p
    biascol = const.tile([128, NKT], F32, name="biascol")
    tri = const.tile([128, 128], F16, name="tri")
    dum_w = const.tile([128, 128], F16, name="dum_w")
    dum_x = const.tile([128, 512], F16, name="dum_x")

    # static init (DVE) -- EA/denominator columns are constant ones now
    nc.vector.memset(dum_w, 0.0)
    nc.vector.memset(dum_x, 0.0)
    nc.vector.memset(v2a[:, :, :, HD:HD + 1], 1.0)
    nc.vector.memset(v2b[:, :, :, 0:1], 1.0)
    nc.vector.memset(v2b[:, :, :, 1:64], 0.0)

    # HAM warm-up: keep the PE clock at 8/8 through the input-DMA phase
    dum_ps = ps_pool.tile([128, 512], F32, tag="ps", name="dum_ps")
    for i in range(16):
        nc.tensor.matmul(dum_ps, dum_w, dum_x, start=True, stop=True)

    # small constants first, then chunk-0 xs interleaved with wqkv c-tiles
    nc.sync.dma_start(out=biascol, in_=biasv.rearrange("(k p) -> p k", p=128))
    nc.sync.dma_start(out=tri, in_=tri_dram[:, :])
    wqkv3 = wqkv.rearrange("(c p) f -> p c f", p=128)

    qts_store = {}
    xs_store = {}

    def gen_chunk(qc):
        """Emit one t-chunk's pre-attention work as resumable items.
        c-innermost matmul order so chunk 0 trickles behind its DMA."""
        xs_tiles = []
        for c in range(NCT):
            xs = xs_pool.tile([128, 512], F16, tag="xs", name=f"xs_{qc}_{c}")
            nc.sync.dma_start(
                out=xs, in_=xT[c * 128:(c + 1) * 128, qc * 512:(qc + 1) * 512]
            )
            xs_tiles.append(xs)
            if qc == 0:
                nc.sync.dma_start(out=wq_sb[:, c, :], in_=wqkv3[:, c, :])
        if qc == 0:
            nc.sync.dma_start(
                out=wp_sb, in_=wproj.rearrange("(i p) c -> p i c", p=128)
            )
        xs_store[qc] = xs_tiles
        yield
        # Q then K: two head-pair halves at a time, c-innermost
        qts = []
        for sec in range(2):          # 0 = Q, 1 = K
            for half in range(2):
                pss = [
                    ps_pool.tile([128, 512], F32, tag="ps",
                                 name=f"qk{sec}_{qc}_{half}_{i}")
                    for i in range(2)
                ]
                for c in range(NCT):
                    for i in range(2):
                        p = 2 * half + i
                        col0 = sec * FPC + p * 128
                        nc.tensor.matmul(
                            pss[i],
                            wq_sb[:, c, col0:col0 + 128],
                            xs_tiles[c],
                            start=(c == 0),
                            stop=(c == NCT - 1),
                        )
                    yield
                for i in range(2):
                    p = 2 * half + i
                    if sec == 0:
                        qt = qt_pool.tile([128, 512], F16, tag="qt",
                                          name=f"qt_{qc}_{p}")
                        nc.vector.tensor_copy(qt, pss[i])
                        qts.append(qt)
                    else:
                        nc.vector.tensor_copy(
                            kt[:, p, qc * 512:(qc + 1) * 512], pss[i]
                        )
                    yield
        qts_store[qc] = qts
        # V: per 128-key block, c-innermost
        for j in range(4):
            kt_i = qc * 4 + j
            ps = ps_pool.tile([128, 512], F32, tag="ps", name=f"vps_{qc}_{j}")
            for c in range(NCT):
                nc.tensor.matmul(
                    ps,
                    xs_tiles[c][:, j * 128:(j + 1) * 128],
                    wq_sb[:, c, 2 * FPC:3 * FPC],
                    start=(c == 0),
                    stop=(c == NCT - 1),
                )
                if c % 2 == 1:
                    yield
            # heads interleave [pair, head-in-pair, dim] in the psum free dim
            ps4 = ps.rearrange("p (h t d) -> p t h d", t=2, d=HD)
            nc.vector.tensor_copy(v2a[:, kt_i, :, 0:HD], ps4[:, 0, :, :])
            nc.vector.tensor_copy(v2b[:, kt_i, :, 64:128], ps4[:, 1, :, :])
            yield

    def gen_proj(qc, stacks, pool, tag):
        for tq in range(4):
            row0 = qc * 512 + tq * 128
            for ch in range(2):
                ps = pool.tile([128, 512], F32, tag=tag,
                               name=f"pps_{qc}_{tq}_{ch}")
                for p in range(NP):
                    nc.tensor.matmul(
                        ps,
                        stacks[p][:, tq * 128:(tq + 1) * 128],
                        wp_sb[:, p, ch * 512:(ch + 1) * 512],
                        start=(p == 0),
                        stop=(p == NP - 1),
                    )
                pout = pout_pool.tile([128, 512], F16, tag="pout",
                                      name=f"po_{qc}_{tq}_{ch}")
                nc.vector.tensor_copy(pout, ps)
                nc.sync.dma_start(
                    out=out_ap[row0:row0 + 128, ch * 512:(ch + 1) * 512],
                    in_=pout,
                )
                yield

    # chunk 0 prework runs up front, paced by its own DMA
    for _ in gen_chunk(0):
        pass

    prev_stacks = None
    for qc in range(NQC):
        nki = 4 * qc + 4
        nslots = NP * nki
        gens = {}
        if qc + 1 < NQC:
            gens["chunk"] = gen_chunk(qc + 1)
        if prev_stacks is not None:
            gens["proj"] = gen_proj(qc - 1, prev_stacks, ps_pool, "ps")
        # proj items are reserved for pair boundaries except what's left
        # over after each boundary spent its share
        proj_left = [8 if "proj" in gens else 0]

        def draw(order, want):
            done = 0
            while done < want:
                g = None
                for k in order:
                    if gens.get(k) is not None:
                        g = k
                        break
                if g is None:
                    return done
                try:
                    next(gens[g])
                except StopIteration:
                    gens[g] = None
                    continue
                if g == "proj":
                    proj_left[0] -= 1
                done += 1
            return done

        def pull(slot, want):
            draw(["chunk"], want)

        def pull_boundary(want):
            draw(["proj", "chunk"], want)

        total_items = 49 if qc + 1 < NQC else 0
        per_slot = max(1, -(-total_items // max(1, nslots - 6)))

        stacks = []
        slot = 0
        qts = qts_store[qc]
        for p in range(NP):
            qt = qts[p]
            yA = y_pool.tile([128, 512], F32, tag="y", name=f"yA_{qc}_{p}")
            yB = y_pool.tile([128, 512], F32, tag="y", name=f"yB_{qc}_{p}")
            for ki in range(nki):
                r = ki - 4 * qc  # >= 0 on the block diagonal
                n0 = 128 * r if r > 0 else 0
                st = st_pool.tile([128, 1024], F32, tag="st",
                                  name=f"st_{qc}_{p}_{ki}")
                st3 = st.rearrange("p (h q) -> p h q", h=2)
                kslice = slice(ki * 128, (ki + 1) * 128)
                nc.tensor.matmul(
                    st3[:, 0, n0:512], kt[0:64, p, kslice], qt[0:64, n0:512],
                    start=True, stop=True,
                )
                nc.tensor.matmul(
                    st3[:, 1, n0:512], kt[64:128, p, kslice],
                    qt[64:128, n0:512],
                    start=True, stop=True,
                )
                se = se_pool.tile([128, 1024], F16, tag="se",
                                  name=f"se_{qc}_{p}_{ki}")
                se3 = se.rearrange("p (h q) -> p h q", h=2)
                nc.scalar.activation(
                    se3[:, :, n0:512], st3[:, :, n0:512], Exp,
                    scale=SCALE, bias=biascol[:, ki:ki + 1],
                )
                if r >= 0:
                    nc.vector.tensor_mul(
                        se3[:, 0, n0:n0 + 128], se3[:, 0, n0:n0 + 128], tri
                    )
                    nc.vector.tensor_mul(
                        se3[:, 1, n0:n0 + 128], se3[:, 1, n0:n0 + 128], tri
                    )
                # filler matmuls land here, covering the exp latency
                pull(slot, per_slot)
                nc.tensor.matmul(
                    yA[0:HD + 1, n0:512], v2a[:, ki, p, :], se3[:, 0, n0:512],
                    start=(ki == 0), stop=(ki == nki - 1),
                    skip_group_check=True,
                )
                nc.tensor.matmul(
                    yB[:, n0:512], v2b[:, ki, p, :], se3[:, 1, n0:512],
                    start=(ki == 0), stop=(ki == nki - 1),
                    skip_group_check=True,
                )
                slot += 1

            # denominators: spread 1024 values over 128 lanes via a DRAM
            # shuffle so the exact reciprocal runs wide, then broadcast back
            idx = qc * 4 + p
            rr = rr_pool.tile([128, 512], F32, tag="rr", name=f"rr_{qc}_{p}")
            nc.vector.tensor_copy(rr[64:65, :], yA[HD:HD + 1, :])
            nc.vector.tensor_copy(rr[0:1, :], yB[0:1, :])
            nc.gpsimd.dma_start(out=dsc[idx, 0:512], in_=rr[64:65, :])
            nc.gpsimd.dma_start(out=dsc[idx, 512:1024], in_=rr[0:1, :])
            dnp = rr_pool.tile([128, 8], F32, tag="dnp", name=f"dnp_{qc}_{p}")
            nc.gpsimd.dma_start(
                out=dnp, in_=dsc[idx, 0:1024].rearrange("(j p) -> p j", p=128)
            )
            rcp = rr_pool.tile([128, 8], F32, tag="rcp", name=f"rcp_{qc}_{p}")
            nc.vector.reciprocal(rcp, dnp)
            nc.gpsimd.dma_start(
                out=dsc[idx, 1024:2048].rearrange("(j p) -> p j", p=128),
                in_=rcp,
            )
            sc = sc_pool.tile([128, 512], F32, tag="sc", name=f"sc_{qc}_{p}")
            nc.gpsimd.dma_start(
                out=sc[0:64, :],
                in_=dsc[idx:idx + 1, 1024:1536].to_broadcast([64, 512]),
            )
            nc.gpsimd.dma_start(
                out=sc[64:128, :],
                in_=dsc[idx:idx + 1, 1536:2048].to_broadcast([64, 512]),
            )
            stack = stack_pool.tile([128, 512], F16, tag="stack",
                                    name=f"stk_{qc}_{p}")
            nc.vector.tensor_mul(stack[0:64, :], yA[0:64, :], sc[0:64, :])
            nc.vector.tensor_mul(stack[64:128, :], yB[64:128, :],
                                 sc[64:128, :])
            stacks.append(stack)
            pull_boundary(3)

        draw(["chunk", "proj"], 1000)
        prev_stacks = stacks

    for _ in gen_proj(NQC - 1, prev_stacks, ps_pool, "ps"):
        pass


def make_nc():
    nc = bacc.Bacc("TRN2", target_bir_lowering=False, debug=False,
                   num_devices=NCORES)
    xT = nc.dram_tensor("xT", [C, T], F16, kind="ExternalInput")
    wqkv = nc.dram_tensor("wqkv", [C, 3 * FPC], F16, kind="ExternalInput")
    wproj = nc.dram_tensor("wproj", [FPC, C], F16, kind="ExternalInput")
    biasv = nc.dram_tensor("biasv", [T], F32, kind="ExternalInput")
    out = nc.dram_tensor("out", [T, C], F16, kind="ExternalOutput")
    dsc = nc.dram_tensor("dsc", [16, 2048], F32, kind="Internal")
    tri_np = np.triu(np.ones((128, 128), dtype=np.float16))
    tri_dram = nc.inline_tensor(tri_np, name="tri_const")
    with ExitStack() as ctx:
        tc = ctx.enter_context(tile.TileContext(nc))
        tc.ctx = ctx
        build(tc, out[:, :], xT[:, :], wqkv[:, :], wproj[:, :], biasv[:],
              tri_dram, dsc[:, :])
    nc.compile()
    return nc


def shard_inputs(x, prev_probs, W_attn, W_proj):
    in_maps = []
    for core in range(NCORES):
        b, g = divmod(core, 2)
        xT = np.ascontiguousarray(x[b].T).astype(np.float16)
        wq = W_attn[:, g * FPC:(g + 1) * FPC]
        wk = W_attn[:, C + g * FPC:C + (g + 1) * FPC]
        wv = W_attn[:, 2 * C + g * FPC:2 * C + (g + 1) * FPC]
        wqkv = np.ascontiguousarray(
            np.concatenate([wq, wk, wv], axis=1)
        ).astype(np.float16)
        wproj = np.ascontiguousarray(
            W_proj[g * FPC:(g + 1) * FPC, :]
        ).astype(np.float16)
        biasv = (EPS_BIAS * -np.log(prev_probs[b] + np.float32(1e-10))
                 ).astype(np.float32)
        in_maps.append(
            {"xT": xT, "wqkv": wqkv, "wproj": wproj, "biasv": biasv}
        )
    return in_maps


_CACHED_NC = None


def kernel(x, prev_probs, W_attn, W_proj, trace=False, tmpdir=None):
    global _CACHED_NC
    from concourse.bass_utils import run_bass_kernel_spmd

    x = np.asarray(x, dtype=np.float32)
    prev_probs = np.asarray(prev_probs, dtype=np.float32)
    W_attn = np.asarray(W_attn, dtype=np.float32)
    W_proj = np.asarray(W_proj, dtype=np.float32)

    if _CACHED_NC is None:
        _CACHED_NC = make_nc()
    nc = _CACHED_NC

    in_maps = shard_inputs(x, prev_probs, W_attn, W_proj)
    res = run_bass_kernel_spmd(
        nc, in_maps, core_ids=list(range(NCORES)), trace=trace, tmpdir=tmpdir
    )
    parts = [r["out"].astype(np.float32) for r in res.results]
    out = np.empty((B, T, C), dtype=np.float32)
    for b in range(B):
        out[b] = parts[2 * b] + parts[2 * b + 1]
    kernel.last_results = res
    return out


# revision 9
# speedup vs baseline: 1.7178x; 1.7178x over previous
"""Trainium2 Bass kernel for CausalSelfAttention (B=4, T=2048, C=1024, H=16)
with additive prev-prob key bias.

Sharding: 8 cores = data-parallel over B (4) x tensor-parallel over head
halves (2).  Each core computes qkv for its 8 heads, causal attention, and a
partial output projection (row-parallel W_proj); host sums the two partials
per batch at unshard time.

v2 design notes (vs the f32r baseline):
  - All matmul operands are fp16 (PSUM accumulation stays f32): halves DMA
    and SBUF traffic, enables FWL weight loads, 2x DVE rate. Measured CPU
    emulation rel-err ~5e-4 (gate is 2e-2).
  - The additive prev-prob bias rides as a per-partition (per-key) bias on
    the exp activation: se = exp(score/8 + 0.1*(-log(p+1e-10))), so V needs
    no per-key scaling and the softmax-denominator column of the PV
    stationary is a constant ones column (memset once).
  - Head B of each pair uses a [128,128] PV stationary (ones col 0 ->
    partition 0 = denominator, dims cols 64:128 -> partitions 64:128) so the
    normalize multiply writes stack[64:128] directly from PSUM -- no SBUF
    assembly DMA and no scalar-engine evacuation copies.
  - Denominator: reciprocal_approx_fast straight off the PSUM row (1-lane
    DVE, ~0.6us), one DMA to DRAM, two broadcast DMAs back (partition
    broadcast requires a DRAM source). ~2.5us chain vs ~9us in the baseline,
    which was the main source of PE stalls + HAM re-throttles.
  - 16 warm-up matmuls on memset tiles keep the PE HAM un-throttled through
    the initial DMA phase; chunk-0 prework matmuls are emitted c-innermost
    so they trickle behind the interleaved xs/wqkv DMA stream.
  - Filler scheduling: next-chunk QKV prework and prev-chunk output
    projection are pulled between a slot's QK and PV (covering the exp
    latency) with an adaptive rate; proj fillers are held back a few slots
    so their LDWEIGHTS never enters the PE queue before the stacks exist.
"""

import math
from contextlib import ExitStack

import numpy as np

import concourse.bass as bass
import concourse.tile as tile
from concourse import bacc, mybir

F32 = mybir.dt.float32
F16 = mybir.dt.float16

B, T, C, H = 4, 2048, 1024, 16
HD = C // H          # 64
NCORES = 8
HPC = H // 2         # 8 heads per core
FPC = HPC * HD       # 512 features per core
NKT = T // 128       # 16 key tiles
NQC = T // 512       # 4 query chunks (also the x t-chunks)
NCT = C // 128       # 8 contraction tiles
NP = HPC // 2        # 4 head pairs
EPS_BIAS = 0.1
SCALE = 1.0 / math.sqrt(HD)


def build(tc, out_ap, xT, wqkv, wproj, biasv, tri_dram, dsc):
    """Emit the per-core kernel into TileContext tc.

    out_ap : (T, C)    fp16 partial projection output (pair-summed on host)
    xT     : (C, T)    fp16 x[b] transposed
    wqkv   : (C, 3*FPC) fp16 [Wq_g | Wk_g | Wv_g] columns for this head group
    wproj  : (FPC, C)  fp16 W_proj rows for this head group
    biasv  : (T,)      f32 0.1 * -log(prev_probs[b] + 1e-10)
    tri_dram: (128,128) fp16 upper-triangular ones (tri[k,q] = 1 iff k <= q)
    dsc    : (16, 2048) f32 DRAM scratch for the denominator shuffle
    """
    nc = tc.nc
    ctx = tc.ctx
    Exp = mybir.ActivationFunctionType.Exp

    const = ctx.enter_context(tc.tile_pool(name="const", bufs=1))
    xs_pool = ctx.enter_context(tc.tile_pool(name="xs", bufs=9))
    qt_pool = ctx.enter_context(tc.tile_pool(name="qt", bufs=9))
    se_pool = ctx.enter_context(tc.tile_pool(name="se", bufs=3))
    rr_pool = ctx.enter_context(tc.tile_pool(name="rr", bufs=3))
    sc_pool = ctx.enter_context(tc.tile_pool(name="sc", bufs=2))
    stack_pool = ctx.enter_context(tc.tile_pool(name="stack", bufs=8))
    pout_pool = ctx.enter_context(tc.tile_pool(name="pout", bufs=3))

    ps_pool = ctx.enter_context(tc.tile_pool(name="ps", bufs=2, space="PSUM"))
    st_pool = ctx.enter_context(tc.tile_pool(name="st", bufs=2, space="PSUM"))
    y_pool = ctx.enter_context(tc.tile_pool(name="y", bufs=2, space="PSUM"))

    # ---- persistent buffers ----
    wq_sb = const.tile([128, NCT, 3 * FPC], F16, name="wq_sb")      # 24KB/p
    wp_sb = const.tile([128, FPC // 128, C], F16, name="wp_sb")     # 8KB/p
    kt = const.tile([128, NP, T], F16, name="kt")                   # 16KB/p
    v2a = const.tile([128, NKT, NP, HD + 1], F16, name="v2a")       # 8.1KB/p
    v2b = const.tile([128, NKT, NP, 128], F16, name="v2b")          # 16KB/p
    biascol = const.tile([128, NKT], F32, name="biascol")
    tri = const.tile([128, 128], F16, name="tri")
    dum_w = const.tile([128, 128], F16, name="dum_w")
    dum_x = const.tile([128, 512], F16, name="dum_x")

    # static init (DVE) -- EA/denominator columns are constant ones now
    nc.vector.memset(dum_w, 0.0)
    nc.vector.memset(dum_x, 0.0)
    nc.vector.memset(v2a[:, :, :, HD:HD + 1], 1.0)
    nc.vector.memset(v2b[:, :, :, 0:1], 1.0)
    nc.vector.memset(v2b[:, :, :, 1:64], 0.0)

    # HAM warm-up: keep the PE clock at 8/8 through the input-DMA phase
    dum_ps = ps_pool.tile([128, 512], F32, tag="ps", name="dum_ps")
    for i in range(16):
        nc.tensor.matmul(dum_ps, dum_w, dum_x, start=True, stop=True)

    # small constants first, then chunk-0 xs interleaved with wqkv c-tiles
    nc.sync.dma_start(out=biascol, in_=biasv.rearrange("(k p) -> p k", p=128))
    nc.sync.dma_start(out=tri, in_=tri_dram[:, :])
    wqkv3 = wqkv.rearrange("(c p) f -> p c f", p=128)

    qts_store = {}
    xs_store = {}

    def gen_chunk(qc):
        """Emit one t-chunk's pre-attention work as resumable items.
        c-innermost matmul order so chunk 0 trickles behind its DMA."""
        xs_tiles = []
        for c in range(NCT):
            xs = xs_pool.tile([128, 512], F16, tag="xs", name=f"xs_{qc}_{c}")
            nc.sync.dma_start(
                out=xs, in_=xT[c * 128:(c + 1) * 128, qc * 512:(qc + 1) * 512]
            )
            xs_tiles.append(xs)
            if qc == 0:
                nc.sync.dma_start(out=wq_sb[:, c, :], in_=wqkv3[:, c, :])
        if qc == 0:
            nc.sync.dma_start(
                out=wp_sb, in_=wproj.rearrange("(i p) c -> p i c", p=128)
            )
        xs_store[qc] = xs_tiles
        yield
        # Q then K: two head-pair halves at a time, c-innermost
        qts = []
        for sec in range(2):          # 0 = Q, 1 = K
            for half in range(2):
                pss = [
                    ps_pool.tile([128, 512], F32, tag="ps",
                                 name=f"qk{sec}_{qc}_{half}_{i}")
                    for i in range(2)
                ]
                for c in range(NCT):
                    for i in range(2):
                        p = 2 * half + i
                        col0 = sec * FPC + p * 128
                        nc.tensor.matmul(
                            pss[i],
                            wq_sb[:, c, col0:col0 + 128],
                            xs_tiles[c],
                            start=(c == 0),
                            stop=(c == NCT - 1),
                        )
                    yield
                for i in range(2):
                    p = 2 * half + i
                    if sec == 0:
                        qt = qt_pool.tile([128, 512], F16, tag="qt",
                                          name=f"qt_{qc}_{p}")
                        nc.vector.tensor_copy(qt, pss[i])
                        qts.append(qt)
                    else:
                        nc.vector.tensor_copy(
                            kt[:, p, qc * 512:(qc + 1) * 512], pss[i]
                        )
                    yield
        qts_store[qc] = qts
        # V: per 128-key block, c-innermost
        for j in range(4):
            kt_i = qc * 4 + j
            ps = ps_pool.tile([128, 512], F32, tag="ps", name=f"vps_{qc}_{j}")
            for c in range(NCT):
                nc.tensor.matmul(
                    ps,
                    xs_tiles[c][:, j * 128:(j + 1) * 128],
                    wq_sb[:, c, 2 * FPC:3 * FPC],
                    start=(c == 0),
                    stop=(c == NCT - 1),
                )
                if c % 2 == 1:
                    yield
            # heads interleave [pair, head-in-pair, dim] in the psum free dim
            ps4 = ps.rearrange("p (h t d) -> p t h d", t=2, d=HD)
            nc.vector.tensor_copy(v2a[:, kt_i, :, 0:HD], ps4[:, 0, :, :])
            nc.vector.tensor_copy(v2b[:, kt_i, :, 64:128], ps4[:, 1, :, :])
            yield

    def gen_proj(qc, stacks, pool, tag):
        for tq in range(4):
            row0 = qc * 512 + tq * 128
            for ch in range(2):
                ps = pool.tile([128, 512], F32, tag=tag,
                               name=f"pps_{qc}_{tq}_{ch}")
                for p in range(NP):
                    nc.tensor.matmul(
                        ps,
                        stacks[p][:, tq * 128:(tq + 1) * 128],
                        wp_sb[:, p, ch * 512:(ch + 1) * 512],
                        start=(p == 0),
                        stop=(p == NP - 1),
                    )
                pout = pout_pool.tile([128, 512], F16, tag="pout",
                                      name=f"po_{qc}_{tq}_{ch}")
                nc.vector.tensor_copy(pout, ps)
                nc.sync.dma_start(
                    out=out_ap[row0:row0 + 128, ch * 512:(ch + 1) * 512],
                    in_=pout,
                )
                yield

    # chunk 0 prework runs up front, paced by its own DMA
    for _ in gen_chunk(0):
        pass

    prev_stacks = None
    for qc in range(NQC):
        nki = 4 * qc + 4
        nslots = NP * nki
        gens = {}
        if qc + 1 < NQC:
            gens["chunk"] = gen_chunk(qc + 1)
        if prev_stacks is not None:
            gens["proj"] = gen_proj(qc - 1, prev_stacks, ps_pool, "ps")
        # proj items are reserved for pair boundaries except what's left
        # over after each boundary spent its share
        proj_left = [8 if "proj" in gens else 0]

        def draw(order, want):
            done = 0
            while done < want:
                g = None
                for k in order:
                    if gens.get(k) is not None:
                        g = k
                        break
                if g is None:
                    return done
                try:
                    next(gens[g])
                except StopIteration:
                    gens[g] = None
                    continue
                if g == "proj":
                    proj_left[0] -= 1
                done += 1
            return done

        def pull(slot, want):
            draw(["chunk"], want)

        def pull_boundary(want):
            draw(["proj", "chunk"], want)

        total_items = 49 if qc + 1 < NQC else 0
        per_slot = max(1, -(-total_items // max(1, nslots - 6)))

        stacks = []
        slot = 0
        qts = qts_store[qc]
        for p in range(NP):
            qt = qts[p]
            yA = y_pool.tile([128, 512], F32, tag="y", name=f"yA_{qc}_{p}")
            yB = y_pool.tile([128, 512], F32, tag="y", name=f"yB_{qc}_{p}")
            for ki in range(nki):
                r = ki - 4 * qc  # >= 0 on the block diagonal
                n0 = 128 * r if r > 0 else 0
                st = st_pool.tile([128, 1024], F32, tag="st",
                                  name=f"st_{qc}_{p}_{ki}")
                st3 = st.rearrange("p (h q) -> p h q", h=2)
                kslice = slice(ki * 128, (ki + 1) * 128)
                nc.tensor.matmul(
                    st3[:, 0, n0:512], kt[0:64, p, kslice], qt[0:64, n0:512],
                    start=True, stop=True,
                )
                nc.tensor.matmul(
                    st3[:, 1, n0:512], kt[64:128, p, kslice],
                    qt[64:128, n0:512],
                    start=True, stop=True,
                )
                se = se_pool.tile([128, 1024], F16, tag="se",
                                  name=f"se_{qc}_{p}_{ki}")
                se3 = se.rearrange("p (h q) -> p h q", h=2)
                nc.scalar.activation(
                    se3[:, :, n0:512], st3[:, :, n0:512], Exp,
                    scale=SCALE, bias=biascol[:, ki:ki + 1],
                )
                if r >= 0:
                    nc.vector.tensor_mul(
                        se3[:, 0, n0:n0 + 128], se3[:, 0, n0:n0 + 128], tri
                    )
                    nc.vector.tensor_mul(
                        se3[:, 1, n0:n0 + 128], se3[:, 1, n0:n0 + 128], tri
                    )
                # filler matmuls land here, covering the exp latency
                pull(slot, per_slot)
                nc.tensor.matmul(
                    yA[0:HD + 1, n0:512], v2a[:, ki, p, :], se3[:, 0, n0:512],
                    start=(ki == 0), stop=(ki == nki - 1),
                    skip_group_check=True,
                )
                nc.tensor.matmul(
                    yB[:, n0:512], v2b[:, ki, p, :], se3[:, 1, n0:512],
                    start=(ki == 0), stop=(ki == nki - 1),
                    skip_group_check=True,
                )
                slot += 1

            # denominators: ACT evacuates the two PSUM rows to partition-0
            # SBUF tiles, approx-reciprocal runs on clean offset-0 APs, then
            # DRAM-bounce partition broadcast
            idx = qc * 4 + p
            da = rr_pool.tile([1, 512], F32, tag="da", name=f"da_{qc}_{p}")
            db = rr_pool.tile([1, 512], F32, tag="db", name=f"db_{qc}_{p}")
            nc.scalar.copy(da, yA[HD:HD + 1, :])
            nc.scalar.copy(db, yB[0:1, :])
            ra = rr_pool.tile([1, 512], F32, tag="ra", name=f"ra_{qc}_{p}")
            rb = rr_pool.tile([1, 512], F32, tag="rb", name=f"rb_{qc}_{p}")
            nc.vector.reciprocal_approx_fast(ra, da)
            nc.vector.reciprocal_approx_fast(rb, db)
            nc.gpsimd.dma_start(out=dsc[idx, 0:512], in_=ra)
            nc.gpsimd.dma_start(out=dsc[idx, 512:1024], in_=rb)
            sc = sc_pool.tile([128, 512], F32, tag="sc", name=f"sc_{qc}_{p}")
            nc.gpsimd.dma_start(
                out=sc[0:64, :],
                in_=dsc[idx:idx + 1, 0:512].to_broadcast([64, 512]),
            )
            nc.gpsimd.dma_start(
                out=sc[64:128, :],
                in_=dsc[idx:idx + 1, 512:1024].to_broadcast([64, 512]),
            )
            stack = stack_pool.tile([128, 512], F16, tag="stack",
                                    name=f"stk_{qc}_{p}")
            nc.vector.tensor_mul(stack[0:64, :], yA[0:64, :], sc[0:64, :])
            nc.vector.tensor_mul(stack[64:128, :], yB[64:128, :],
                                 sc[64:128, :])
            stacks.append(stack)
            pull_boundary(3)

        draw(["chunk", "proj"], 1000)
        prev_stacks = stacks

    for _ in gen_proj(NQC - 1, prev_stacks, ps_pool, "ps"):
        pass


def make_nc():
    nc = bacc.Bacc("TRN2", target_bir_lowering=False, debug=False,
                   num_devices=NCORES)
    xT = nc.dram_tensor("xT", [C, T], F16, kind="ExternalInput")
    wqkv = nc.dram_tensor("wqkv", [C, 3 * FPC], F16, kind="ExternalInput")
    wproj = nc.dram_tensor("wproj", [FPC, C], F16, kind="ExternalInput")
    biasv = nc.dram_tensor("biasv", [T], F32, kind="ExternalInput")
    out = nc.dram_tensor("out", [T, C], F16, kind="ExternalOutput")
    dsc = nc.dram_tensor("dsc", [16, 2048], F32, kind="Internal")
    tri_np = np.triu(np.ones((128, 128), dtype=np.float16))
    tri_dram = nc.inline_tensor(tri_np, name="tri_const")
    with ExitStack() as ctx:
        tc = ctx.enter_context(tile.TileContext(nc))
        tc.ctx = ctx
        build(tc, out[:, :], xT[:, :], wqkv[:, :], wproj[:, :], biasv[:],
              tri_dram, dsc[:, :])
    nc.compile()
    return nc


def shard_inputs(x, prev_probs, W_attn, W_proj):
    in_maps = []
    for core in range(NCORES):
        b, g = divmod(core, 2)
        xT = np.ascontiguousarray(x[b].T).astype(np.float16)
        wq = W_attn[:, g * FPC:(g + 1) * FPC]
        wk = W_attn[:, C + g * FPC:C + (g + 1) * FPC]
        wv = W_attn[:, 2 * C + g * FPC:2 * C + (g + 1) * FPC]
        wqkv = np.ascontiguousarray(
            np.concatenate([wq, wk, wv], axis=1)
        ).astype(np.float16)
        wproj = np.ascontiguousarray(
            W_proj[g * FPC:(g + 1) * FPC, :]
        ).astype(np.float16)
        biasv = (EPS_BIAS * -np.log(prev_probs[b] + np.float32(1e-10))
                 ).astype(np.float32)
        in_maps.append(
            {"xT": xT, "wqkv": wqkv, "wproj": wproj, "biasv": biasv}
        )
    return in_maps


_CACHED_NC = None


def kernel(x, prev_probs, W_attn, W_proj, trace=False, tmpdir=None):
    global _CACHED_NC
    from concourse.bass_utils import run_bass_kernel_spmd

    x = np.asarray(x, dtype=np.float32)
    prev_probs = np.asarray(prev_probs, dtype=np.float32)
    W_attn = np.asarray(W_attn, dtype=np.float32)
    W_proj = np.asarray(W_proj, dtype=np.float32)

    if _CACHED_NC is None:
        _CACHED_NC = make_nc()
    nc = _CACHED_NC

    in_maps = shard_inputs(x, prev_probs, W_attn, W_proj)
    res = run_bass_kernel_spmd(
        nc, in_maps, core_ids=list(range(NCORES)), trace=trace, tmpdir=tmpdir
    )
    parts = [r["out"].astype(np.float32) for r in res.results]
    out = np.empty((B, T, C), dtype=np.float32)
    for b in range(B):
        out[b] = parts[2 * b] + parts[2 * b + 1]
    kernel.last_results = res
    return out
